# revision 14
# baseline (speedup 1.0000x reference)
"""Trainium2 Bass kernel for nn_ALAttention (sparse local attention), v2.

Sharding: 64 image rows split across 8 cores (8 query rows each); 16-row halo
slab of x per core (zero-padded at borders), identical SPMD graph, no
inter-core communication.

v2 redesign vs baseline:
- Banded attention: each 128-query block (2 image rows) attends to exactly 5
  key chunks (10 slab rows) instead of 6.5 dense chunks -> less PE/ACT/DVE.
- V^T produced directly by a transposed GEMM (lhsT = x chunk, rhs = w_v),
  eliminating all PE transposes and PSUM->SBUF v copies.
- K bias dropped (constant-per-query score shift, softmax-invariant);
  V bias folded into the proj bias on host (softmax weights sum to 1).
- Normalization: ones-column in V^T yields denominators; reciprocal on DVE,
  partition-broadcast via a tiny f32r PE matmul (ones2 lhsT), fused scale
  into the PSUM->SBUF ocat move.
- Fine-grained schedule: attention units (pair, block, eo) pipelined with
  AV delayed 2 units; batch-1 QKV/V^T GEMMs and batch-0 proj emitted as PE
  fillers inside the attention stream to keep the PE dense (full p-state).
"""
import os
import sys
import types
from collections import deque

sys.path.insert(0, "/opt/trn_rl_repo")

import numpy as np
import ml_dtypes

from concourse import bacc, tile, mybir
from concourse import bass_utils
from concourse.bass_utils import run_bass_kernel_spmd

F32 = mybir.dt.float32
F32R = mybir.dt.float32r
BF16 = mybir.dt.bfloat16
AF = mybir.ActivationFunctionType
ALU = mybir.AluOpType

B = 2
C = 384
HH = WW = 64
HEADS = 6
NCORES = 8
ROWS = 8
SLAB = 16
SCOLS = SLAB * WW      # 1024 slab key positions
QCOLS = ROWS * WW      # 512 queries per core
NKC = SCOLS // 128     # 8 key chunks
NBLK = 4               # query blocks of 128 (2 image rows each)
# uniform 5-chunk band: block p covers key chunks p..p+4. Border cores get a
# host-side slab-row remap (duplicating the out-of-band rows into the unused
# padding slots) so the same band covers their clamped attention reach.
CS = (0, 1, 2, 3)
NB = (5, 5, 5, 5)
# chunk-major: chunk ch serves query blocks B0[ch]..B1[ch] (contiguous)
CB0 = tuple(max(0, ch - 4) for ch in range(8))
CW = tuple((min(3, ch) - max(0, ch - 4) + 1) * 128 for ch in range(8))
MOFF = (0,)
for _ch in range(1, 9):
    MOFF = MOFF + (MOFF[-1] + 2 * CW[_ch - 1],)
MTOT = MOFF[8]
SCALE = float(64) ** -0.5

LAST_EXEC_NS = None
LAST_TRACE = None
_NC_CACHE = {}


def _register_ntff_hook():
    if "antenv.axon_hooks" in sys.modules:
        return
    try:
        from trn_agent_boot.trn_boot import _ntff_profile_via_ctypes
        hook = _ntff_profile_via_ctypes("/opt/axon/libaxon_pjrt.so")
    except Exception:
        hook = None
    mod = types.ModuleType("antenv.axon_hooks")
    mod.get_axon_ntff_profile_hook = lambda: hook
    mod.set_axon_ntff_profile_hook = lambda h: None
    sys.modules["antenv.axon_hooks"] = mod
    bass_utils.upload_artifacts = lambda tmpdir: "local://skipped"


def build_graph():
    nc = bacc.Bacc("TRN2", target_bir_lowering=False, debug=False,
                   num_devices=NCORES)

    xs_e = nc.dram_tensor("xs", [B, C, SCOLS], BF16, kind="ExternalInput").ap()
    wqkvT_e = nc.dram_tensor("wqkvT", [C, 3 * C], BF16,
                             kind="ExternalInput").ap()
    bq_e = nc.dram_tensor("bq", [128, 3], F32, kind="ExternalInput").ap()
    wprojT_e = nc.dram_tensor("wprojT", [C, C], BF16, kind="ExternalInput").ap()
    bproj_e = nc.dram_tensor("bproj", [128, 3], F32, kind="ExternalInput").ap()
    mask_e = nc.dram_tensor("mask", [128, MTOT], BF16,
                            kind="ExternalInput").ap()
    out_e = nc.dram_tensor("out", [B, C, QCOLS], F32, kind="ExternalOutput").ap()

    with tile.TileContext(nc) as tc:
        with (
            tc.tile_pool(name="const", bufs=1) as cpool,
            tc.tile_pool(name="xin", bufs=2) as xpool,
            tc.tile_pool(name="qkv", bufs=2) as qkvpool,
            tc.tile_pool(name="esb", bufs=10) as epool,
            tc.tile_pool(name="oc", bufs=2) as ocpool,
            tc.tile_pool(name="sc", bufs=3) as scpool,
            tc.tile_pool(name="psQ", bufs=2, space="PSUM") as psQ,
            tc.tile_pool(name="psS", bufs=4, space="PSUM") as psS,
            tc.tile_pool(name="psO", bufs=2, space="PSUM") as psO,
        ):
            # ---- input DMAs: x b0 (per-k, per-column-half) + w(qk) first so
            # the first K GEMM is gated only on its own pieces ----
            qs = [nc.sync, nc.scalar, nc.gpsimd]
            w_sb = cpool.tile([128, 3, 3 * C], BF16, tag="wqkv")
            x_sb = {}
            x_sb[0] = xpool.tile([128, 3, SCOLS], BF16, tag="x", name="x_b0")
            # one critical DMA per sequencer first; whole-chunk transfers
            nc.sync.dma_start(w_sb[:, 0, 0:768], wqkvT_e[0:128, 0:768])
            nc.scalar.dma_start(x_sb[0][:, 0, :], xs_e[0, 0:128, :])
            nc.gpsimd.dma_start(w_sb[:, 1, 0:768], wqkvT_e[128:256, 0:768])
            nc.sync.dma_start(x_sb[0][:, 1, :], xs_e[0, 128:256, :])
            nc.scalar.dma_start(w_sb[:, 2, 0:768], wqkvT_e[256:384, 0:768])
            nc.gpsimd.dma_start(x_sb[0][:, 2, :], xs_e[0, 256:384, :])
            bq_sb = cpool.tile([128, 3], F32, tag="bq")
            nc.sync.dma_start(bq_sb[:], bq_e[:])
            qs2 = [nc.sync, nc.scalar, nc.gpsimd]
            for k in range(3):
                qs2[k].dma_start(w_sb[:, k, 768:1152],
                                 wqkvT_e[128 * k:128 * (k + 1), 768:1152])
            mask_sb = cpool.tile([128, MTOT], BF16, tag="mask")
            half = MTOT // 2
            nc.sync.dma_start(mask_sb[:, 0:half], mask_e[:, 0:half])
            nc.scalar.dma_start(mask_sb[:, half:MTOT], mask_e[:, half:MTOT])
            x_sb[1] = xpool.tile([128, 3, SCOLS], BF16, tag="x", name="x_b1")
            for k in range(3):
                qs2[k].dma_start(x_sb[1][:, k, :],
                                 xs_e[1, 128 * k:128 * (k + 1), :])
            wp_sb = cpool.tile([128, 3, C], BF16, tag="wproj")
            for k in range(3):
                qs2[k].dma_start(wp_sb[:, k, :],
                                 wprojT_e[128 * k:128 * (k + 1), :])
            bp_sb = cpool.tile([128, 3], F32, tag="bproj")
            nc.sync.dma_start(bp_sb[:], bproj_e[:])

            # pre-warm the EXP table during the QKV phase
            warm_sb = cpool.tile([1, 1], F32, tag="warm")
            nc.scalar.activation(warm_sb[:], bq_sb[0:1, 0:1], AF.Exp)
            # PE p-state warm-up: dummy matmuls on const data while the
            # first x/w DMAs land
            dummy = cpool.tile([128, 512], BF16, tag="dummy")
            nc.gpsimd.memset(dummy[:], 0.25)
            zeros65 = cpool.tile([128, 65], BF16, tag="z65")
            nc.gpsimd.memset(zeros65[:], 0.0)
            wps = psQ.tile([128, 512], F32, tag="q", name="warmup_ps")
            for _ in range(8):
                nc.tensor.matmul(wps[:], dummy[:, 0:128], dummy[:],
                                 start=True, stop=True)

            q_sb = {b: qkvpool.tile([128, 3, QCOLS], BF16, tag="q",
                                    name=f"q_sb{b}") for b in range(B)}
            k_sb = {b: qkvpool.tile([128, 3, SCOLS], BF16, tag="k",
                                    name=f"k_sb{b}") for b in range(B)}
            vT_sb = {b: qkvpool.tile([128, NKC, HEADS, 65], BF16, tag="vt",
                                     name=f"vT_sb{b}") for b in range(B)}

            uid = [0]

            def nm(s):
                uid[0] += 1
                return f"{s}_{uid[0]}"

            def k_gemm(b, c, h):
                ps = psQ.tile([128, 512], F32, tag="q", name=nm("psk"))
                for k in range(3):
                    nc.tensor.matmul(
                        ps[:], w_sb[:, k, 384 + 128 * c:384 + 128 * (c + 1)],
                        x_sb[b][:, k, 512 * h:512 * (h + 1)],
                        start=(k == 0), stop=(k == 2))
                nc.vector.tensor_copy(k_sb[b][:, c, 512 * h:512 * (h + 1)],
                                      ps[:])

            def q_gemm(b, c):
                ps = psQ.tile([128, 512], F32, tag="q", name=nm("psq"))
                for k in range(3):
                    nc.tensor.matmul(
                        ps[:], w_sb[:, k, 128 * c:128 * (c + 1)],
                        x_sb[b][:, k, 256:768],
                        start=(k == 0), stop=(k == 2))
                nc.scalar.activation(q_sb[b][:, c, :], ps[:], AF.Identity,
                                     bias=bq_sb[:, c:c + 1], scale=1.0)

            def vT_gemm(b, j):
                ps = psQ.tile([128, 6, 64], F32, tag="q", name=nm("psv"))
                for k in range(3):
                    nc.tensor.matmul(
                        ps[:], x_sb[b][:, k, 128 * j:128 * (j + 1)],
                        w_sb[:, k, 768:1152],
                        start=(k == 0), stop=(k == 2))
                nc.scalar.copy(vT_sb[b][:, j, :, 0:64], ps[:])
                nc.gpsimd.memset(vT_sb[b][:, j, :, 64:65], 1.0)

            # ---- attention units ----
            ot_tiles = {}

            def score_unit(b, c, ch):
                w = CW[ch]
                g0 = 128 * CB0[ch]
                es = []
                for eo in range(2):
                    st = psS.tile([128, w], F32, tag="st", name=nm("st"))
                    nc.tensor.matmul(
                        st[:],
                        k_sb[b][64 * eo:64 * eo + 64, c,
                                128 * ch:128 * (ch + 1)],
                        q_sb[b][64 * eo:64 * eo + 64, c, g0:g0 + w],
                        start=True, stop=True)
                    e = epool.tile([128, w], BF16, tag="e", name=nm("e"))
                    nc.scalar.activation(e[:], st[:], AF.Exp)
                    nc.vector.tensor_tensor(
                        e[:], e[:],
                        mask_sb[:, MOFF[ch] + eo * w:MOFF[ch] + (eo + 1) * w],
                        ALU.mult)
                    es.append(e)
                return es

            def av_unit(b, c, ch, es):
                w = CW[ch]
                g0 = 128 * CB0[ch]
                for eo in range(2):
                    e = es[eo]
                    if (b, c, eo) not in ot_tiles:
                        ot = psO.tile([65, QCOLS], F32, tag="ot",
                                      name=nm("ot"))
                        ot_tiles[(b, c, eo)] = ot
                        # start=True lazily zeroes the whole bank; write a
                        # zero product so all later AVs accumulate
                        nc.tensor.matmul(ot[:], zeros65[:], dummy[:],
                                         start=True, stop=False,
                                         skip_group_check=True)
                    ot = ot_tiles[(b, c, eo)]
                    h = 2 * c + eo
                    nc.tensor.matmul(
                        ot[:, g0:g0 + w], vT_sb[b][:, ch, h, :], e[:, 0:w],
                        start=False, stop=(ch == NKC - 1),
                        skip_group_check=True)

            ocats = {}

            def norm_eo(b, c, eo):
                ot = ot_tiles[(b, c, eo)]
                srow = scpool.tile([1, QCOLS], F32, tag=f"sr{eo}",
                                   name=nm("sr"))
                nc.vector.tensor_copy(srow[:], ot[64:65, :])
                rr = scpool.tile([1, QCOLS], F32, tag=f"rr{eo}", name=nm("rr"))
                nc.vector.reciprocal_approx_fast(rr[:], srow[:])
                rb = scpool.tile([64, QCOLS], F32, tag=f"rb{eo}", name=nm("rb"))
                nc.gpsimd.partition_broadcast(rb[:], rr[:])
                if (b, c) not in ocats:
                    ocats[(b, c)] = ocpool.tile([128, QCOLS], BF16,
                                                tag=f"oc{c}", name=nm("oc"))
                oc = ocats[(b, c)]
                nc.vector.tensor_tensor(oc[64 * eo:64 * eo + 64, :],
                                        ot[0:64, :], rb[:], ALU.mult)

            def proj_m(b, m):
                if b == 0:
                    pp = psQ.tile([128, 512], F32, tag="q", name=nm("pp"))
                else:
                    pp = psS.tile([128, 512], F32, tag="st", name=nm("pp"))
                for k in range(3):
                    nc.tensor.matmul(
                        pp[:], wp_sb[:, k, 128 * m:128 * (m + 1)],
                        ocats[(b, k)][:],
                        start=(k == 0), stop=(k == 2))
                o_sb = scpool.tile([128, 512], F32, tag="o", name=nm("o"))
                if b == 0:
                    nc.vector.tensor_scalar(o_sb[:], pp[:], bp_sb[:, m:m + 1],
                                            None, ALU.add)
                else:
                    nc.scalar.activation(o_sb[:], pp[:], AF.Identity,
                                         bias=bp_sb[:, m:m + 1], scale=1.0)
                qs2[m].dma_start(out_e[b, 128 * m:128 * (m + 1), :], o_sb[:])

            pp_b1 = {}

            def proj_b1_partial(m):
                pp = psQ.tile([128, 512], F32, tag="q", name=nm("ppb"))
                for k in range(2):
                    nc.tensor.matmul(
                        pp[:], wp_sb[:, k, 128 * m:128 * (m + 1)],
                        ocats[(1, k)][:], start=(k == 0), stop=False,
                        skip_group_check=True)
                pp_b1[m] = pp

            def proj_b1_final(m):
                if m in pp_b1:
                    pp = pp_b1[m]
                    nc.tensor.matmul(
                        pp[:], wp_sb[:, 2, 128 * m:128 * (m + 1)],
                        ocats[(1, 2)][:], start=False, stop=True,
                        skip_group_check=True)
                else:
                    pp = psS.tile([128, 512], F32, tag="st", name=nm("ppf"))
                    for k in range(3):
                        nc.tensor.matmul(
                            pp[:], wp_sb[:, k, 128 * m:128 * (m + 1)],
                            ocats[(1, k)][:], start=(k == 0), stop=(k == 2))
                o_sb = scpool.tile([128, 512], F32, tag="o", name=nm("o"))
                nc.scalar.activation(o_sb[:], pp[:], AF.Identity,
                                     bias=bp_sb[:, m:m + 1], scale=1.0)
                qs2[m].dma_start(out_e[1, 128 * m:128 * (m + 1), :], o_sb[:])

            # ---- one continuous stream: attention units with all QKV/V^T
            # GEMMs as dependency-ordered fillers ----
            fillers = deque()
            for b in range(B):
                for c in range(3):
                    fillers.append((("k", b, c, 0),
                                    lambda b=b, c=c: k_gemm(b, c, 0)))
                    fillers.append((("k", b, c, 1),
                                    lambda b=b, c=c: k_gemm(b, c, 1)))
                    fillers.append((("q", b, c), lambda b=b, c=c: q_gemm(b, c)))
                    if c == 0:
                        for j in range(5):
                            fillers.append((("vT", b, j),
                                            lambda b=b, j=j: vT_gemm(b, j)))
                    elif c == 1:
                        for j in range(5, NKC):
                            fillers.append((("vT", b, j),
                                            lambda b=b, j=j: vT_gemm(b, j)))
            done = set()

            def ensure(key):
                while key not in done:
                    k2, fn = fillers.popleft()
                    fn()
                    done.add(k2)

            def pop_filler():
                if fillers:
                    k2, fn = fillers.popleft()
                    fn()
                    done.add(k2)

            units = [(b, c, ch) for b in range(B) for c in range(3)
                     for ch in range(NKC)]
            pending = deque()

            def retire_one():
                (b, c, ch), e = pending.popleft()
                ensure(("vT", b, ch))
                av_unit(b, c, ch, e)
                if ch == NKC - 1:
                    norm_eo(b, c, 0)
                    norm_eo(b, c, 1)
                    if b == 0 and c == 2:
                        for m in range(3):
                            fillers.append(
                                (("proj", 0, m), lambda m=m: proj_m(0, m)))
                    if b == 1 and c == 1:
                        fillers.append((("ppb", 0), lambda: proj_b1_partial(0)))
                        fillers.append((("ppb", 1), lambda: proj_b1_partial(1)))

            for i, u in enumerate(units):
                b, c, ch = u
                ensure(("k", b, c, 1 if ch >= 4 else 0))
                ensure(("q", b, c))
                e = score_unit(*u)
                pending.append((u, e))
                thresh = 2 if i < len(units) - 4 else 1
                while len(pending) > thresh:
                    retire_one()
                pop_filler()
            while pending:
                retire_one()
            while fillers:
                pop_filler()
            # keep the PE warm through the last normalize chain
            wps2 = psS.tile([128, 512], F32, tag="st", name="warm2_ps")
            for _ in range(6):
                nc.tensor.matmul(wps2[:], dummy[:, 0:128], dummy[:],
                                 start=True, stop=True)
            for m in range(3):
                proj_b1_final(m)

    nc.compile()
    return nc


def _build_inputs(x, w_qkv, b_qkv, w_proj, b_proj, attn_idx):
    bf = ml_dtypes.bfloat16
    x = np.asarray(x, np.float32)
    w_qkv = np.asarray(w_qkv, np.float32)
    b_qkv = np.asarray(b_qkv, np.float32)
    w_proj = np.asarray(w_proj, np.float32)
    b_proj = np.asarray(b_proj, np.float32)
    attn_idx = np.asarray(attn_idx)

    xp = np.zeros((B, C, HH + 8, WW), np.float32)
    xp[:, :, 4:4 + HH, :] = x
    xp = xp.astype(bf)

    wqkvT = np.ascontiguousarray(w_qkv.T)
    wqkvT[:, :C] *= SCALE  # fold q scale into weights
    wqkvT = wqkvT.astype(bf)
    wprojT = np.ascontiguousarray(w_proj.T).astype(bf)

    # q bias prescaled by SCALE (applied via activation scale on the q GEMM)
    bq = np.ascontiguousarray((b_qkv[:C] * SCALE).reshape(3, 128).T)
    # v bias folded into proj bias (softmax weights sum to 1); k bias dropped
    # (constant shift per query, softmax-invariant)
    bproj_eff = b_proj + w_proj @ b_qkv[2 * C:3 * C]
    bproj = np.ascontiguousarray(bproj_eff.reshape(3, 128).T)

    in_maps = []
    for i in range(NCORES):
        # slab rows as xp-row indices; border cores duplicate the rows their
        # clamped attention needs into the unused padding slots so the
        # uniform block-band (chunks p..p+4) covers them
        if i == 0:
            rows = [0, 10, 11, 12] + list(range(4, 16))
        elif i == NCORES - 1:
            base = 8 * i
            rows = [base + r for r in
                    list(range(0, 12)) + [4, 3, 5, 12]]
        else:
            rows = list(range(8 * i, 8 * i + SLAB))
        slab = np.ascontiguousarray(
            xp[:, :, rows, :]).reshape(B, C, SCOLS)
        img_of_slab = [xr + (0 if i == 0 else 8 * i) - 4 if False else None
                       for xr in rows]
        img_of_slab = [xr - 4 for xr in rows]  # xp row -> image row (pads <0 or >63)
        q0 = 8 * i * WW
        gq = np.arange(q0, q0 + QCOLS)
        aidx = attn_idx[gq].astype(np.int64)
        # sanity: queries sit at slab rows 4..11
        assert img_of_slab[4:12] == list(range(8 * i, 8 * i + 8)), \
            f"core {i}: query rows displaced by remap"
        m = np.zeros((128, MTOT), np.float32)
        for p in range(NBLK):
            # image row -> slab row within this block's band (first match)
            band = range(2 * p, 2 * p + 10)
            row_at = {}
            for s in band:
                row_at.setdefault(img_of_slab[s], s)
            aq = aidx[128 * p:128 * (p + 1)]
            for q in range(128):
                for t in aq[q]:
                    R, cc = int(t) // WW, int(t) % WW
                    assert R in row_at, \
                        f"core {i} block {p}: target row {R} not in band"
                    s = row_at[R]
                    pos = s * WW + cc
                    ch = pos // 128
                    j = 128 * p + q - 128 * CB0[ch]
                    w = CW[ch]
                    for eo in range(2):
                        m[pos % 128, MOFF[ch] + eo * w + j] = 1.0
        in_maps.append({
            "xs": slab,
            "wqkvT": wqkvT,
            "bq": bq,
            "wprojT": wprojT,
            "bproj": bproj,
            "mask": np.ascontiguousarray(m).astype(bf),
        })
    return in_maps


def kernel(x, w_qkv, b_qkv, w_proj, b_proj, attn_idx):
    global LAST_EXEC_NS, LAST_TRACE
    _register_ntff_hook()
    if "graph" not in _NC_CACHE:
        _NC_CACHE["graph"] = build_graph()
    nc = _NC_CACHE["graph"]
    in_maps = _build_inputs(x, w_qkv, b_qkv, w_proj, b_proj, attn_idx)
    trace = bool(int(os.environ.get("BASSK_TRACE", "0")))
    res = run_bass_kernel_spmd(nc, in_maps, core_ids=list(range(NCORES)),
                               trace=trace)
    LAST_EXEC_NS = res.exec_time_ns
    if res.instructions_and_trace is not None:
        LAST_TRACE = res.instructions_and_trace[1]
    out = np.empty((B, C, HH, WW), np.float32)
    for i in range(NCORES):
        o = res.results[i]["out"].reshape(B, C, ROWS, WW)
        out[:, :, 8 * i:8 * i + ROWS, :] = o
    return out
